# revision 30
# baseline (speedup 1.0000x reference)
"""ARAP cell-energy Bass kernel (per-core SPMD program), v2.

Problem: B=2, N=8192, K=16. 8 cores; core c owns rows [c*1024,(c+1)*1024)
of BOTH batches -> 2048 points/core. Point mapping: pt = q*128 + p
(q in [0,16) slot-group, p = partition). Natural shard row order == pt.

Gather strategy (v2):
  - coords: host uploads 6 coordinate "planes" [6,16384] f32; device
    replicates them across all 128 partitions (partition 16g+j holds
    plane j), then ONE gpsimd.ap_gather fetches all 32768 neighbor
    coords (per-16-partition-group index lists, ~0.4us), and 32 PE
    transposes + DVE copies land them point-major in gtab (8-f32 slots).
  - weights: 32 x dma_gather (1024 idx each, 4 SWDGE queues -> 4x
    faster descriptor gen) fetch 256B windows wmat[pt, (n>>6)*64:+64];
    a one-hot (iota==n%64) mask + multiply + reduce extracts w[pt,n].
  - rotation: Frobenius-normalized Newton-Schulz, 6 quintic + 3 cubic
    iterations, det flip via 2-step power iteration on adj(Q^T X0).

Host-side prep in shard_inputs() is pure layout/index arithmetic
(transposes, reshapes, int16 casts, shifts); all data movement and
compute stay on device.
"""
import numpy as np
import concourse.bass as bass
import concourse.bacc as bacc
import concourse.tile as tile
import concourse.mybir as mybir
from concourse import ap_utils
from concourse.bass import exact_div

F32 = mybir.dt.float32
I32 = mybir.dt.int32
I16 = mybir.dt.int16
ALU = mybir.AluOpType
ACT = mybir.ActivationFunctionType
AXL = mybir.AxisListType

B, N, K = 2, 8192, 16
NCORES = 8
RPC = N // NCORES          # rows per core per batch = 1024
PTS = B * RPC              # points per core = 2048
P = 128                    # partitions
PPT = PTS // P             # slot-groups q = 16
NSLOT = PPT * K            # slots per partition = 256
EPS_S = 1e-8 * 1e-4

QA, QB, QC = 3.4445, -4.7750, 2.0315
N_QUINTIC = 5
N_CUBIC = 3
N_POWER = 2

NW_INSTR = 32              # weight gather instructions (1024 idx each)
WCHUNK = 8                 # weight instrs per extraction chunk
NCHUNK = NW_INSTR // WCHUNK


def v(ap, off, dims):
    part = list(ap.ap[0])
    return bass.AP(ap.tensor, ap.offset + off, [part] + [[s, n] for s, n in dims])


def dma_gather_raw(gp, out_ap, in_ap, idxs_ap, num_idxs, elem_size, elem_step,
                   queue_num=0):
    """dma_gather without the elem_size%256 restriction (non-transpose,
    DRAM source)."""
    assert idxs_ap.dtype == I16
    assert in_ap.dtype == out_ap.dtype
    assert in_ap.space == bass.MemorySpace.DRAM
    assert ap_utils.ap_is_contiguous(out_ap.ap[1:])
    assert ap_utils.ap_is_contiguous(idxs_ap.ap[1:])
    assert in_ap.ap[0][0] == elem_step
    stride_bytes_256 = exact_div(elem_step * mybir.dt.size(in_ap.dtype), 256)
    _in_ap = gp.lower_ap_dma(in_ap, for_custom_bir_dma=True)
    return gp.add_instruction(
        mybir.InstDMAGatherAnt(
            name=gp.bass.get_next_instruction_name(),
            ins=[*_in_ap, gp.lower_ap(idxs_ap),
                 gp.lower_val_access(gp.to_reg(num_idxs))],
            outs=[gp.lower_ap(out_ap)],
            transpose=False, num_idxs=num_idxs, elem_size=elem_size,
            stride_bytes_256=stride_bytes_256, gen_mode=0, single_packet=True,
            queue_num=queue_num, sbuf_tokens_per_rank=0,
            sbuf_free_dim_per_rank=0, sbuf_free_dim_pad_per_rank=0,
            sbuf_byte_offset=0))


def build(nc):
    planes = nc.declare_dram_parameter("planes", [6, B * N], F32, isOutput=False)
    aidx = nc.declare_dram_parameter("aidx", [P, NSLOT], I16, isOutput=False)
    widx = nc.declare_dram_parameter("widx", [P, 2048], I16, isOutput=False)
    misc = nc.declare_dram_parameter("misc", [P, 496], F32, isOutput=False)
    wmat = nc.declare_dram_parameter("wmat", [PTS, N], F32, isOutput=False)
    out = nc.declare_dram_parameter("out", [P, PPT * 3], F32, isOutput=True)
    with tile.TileContext(nc) as tc:
        with tc.tile_pool(name="sb", bufs=1) as pool, \
             tc.tile_pool(name="gw", bufs=2) as gwpool, \
             tc.tile_pool(name="msk", bufs=1) as mpool, \
             tc.tile_pool(name="ps", bufs=8, space="PSUM") as pspool:
            # ---------------- small loads ----------------
            aidx_sb = pool.tile([P, NSLOT], I16)
            widx_sb = pool.tile([P, 2048], I16)
            misc_sb = pool.tile([P, 496], F32)
            nc.sync.dma_start(widx_sb[:], widx[:])
            nc.scalar.dma_start(aidx_sb[:], aidx[:])
            nc.scalar.dma_start(misc_sb[:], misc[:])
            n63_sb = misc_sb      # [:, 0:256]
            ident_sb = misc_sb    # [:, 256:384]
            own1_sb = misc_sb     # [:, 400:448]
            own2_sb = misc_sb     # [:, 448:496]

            # ---------------- plane table, replicated ----------------
            # Everything ap_gather needs rides the Act HWDGE queue
            # (~33GB/s, otherwise idle). The SP queue is starved during the
            # weight-window phase, so anything queued there lands ~100us
            # late; it keeps only widx (consumed progressively by the
            # gathers) and the output.
            ptab = pool.tile([P, B * N], F32)
            for g in range(8):
                nc.scalar.dma_start(ptab[16 * g:16 * g + 6, :], planes[:])

            # ---------------- coord gather (ap_gather) ----------------
            ga = pool.tile([P, NSLOT * 16], F32)  # [p, 4096] per-group slots
            nc.gpsimd.ap_gather(ga[:], ptab[:], aidx_sb[:],
                                channels=P, num_elems=B * N, d=1,
                                num_idxs=4096)
            # transpose back to point-major gtab (8-f32 slots, 6 used)
            gtab = pool.tile([P, NSLOT * 8], F32)
            for b in range(32):
                ps = pspool.tile([P, P], F32)
                nc.tensor.transpose(out=ps[:], in_=ga[:, P * b:P * (b + 1)],
                                    identity=ident_sb[:, 256:384])
                base = ((b >> 4) * 16 + (b & 15)) * 8
                nc.vector.tensor_copy(
                    out=v(gtab[:], base, [(256, 8), (1, 6)]),
                    in_=v(ps[:], 0, [(16, 8), (1, 6)]))

            # ---------------- weight window gathers (4 SWDGE queues) ----
            gw_tiles = []
            for c in range(NCHUNK):
                gw = gwpool.tile([P, WCHUNK * 8 * 64], F32)
                gw_tiles.append(gw)
                for t in range(WCHUNK):
                    j = c * WCHUNK + t
                    q = j // 2
                    in_ap = bass.AP(wmat[:].tensor, q * 128 * N,
                                    [[64, 128 * 128], [1, 64]])
                    out_ap = bass.AP(gw[:].tensor, gw[:].offset + t * 8 * 64,
                                     [list(gw[:].ap[0]), [64, 8], [1, 64]])
                    nc.gpsimd.dma_gather(out_ap, in_ap,
                                         widx_sb[:, 64 * j:64 * (j + 1)],
                                         1024, 1024, 64, queue_num=j % 4)

            # ---------------- weight extraction ----------------
            iota64i = pool.tile([P, 64], I32)
            with tc.high_priority():
                nc.gpsimd.iota(iota64i[:], pattern=[[1, 64]], base=0,
                               channel_multiplier=0)
            iota64 = pool.tile([P, 64], F32)
            nc.vector.tensor_copy(out=iota64[:], in_=iota64i[:])
            wg = pool.tile([P, NSLOT], F32)
            for c in range(NCHUNK):
                gw = gw_tiles[c]
                emask = mpool.tile([P, 64 * 64], F32)
                nc.vector.tensor_tensor(
                    out=emask[:],
                    in0=v(iota64[:], 0, [(0, 64), (1, 64)]),
                    in1=v(n63_sb[:], 64 * c, [(1, 64), (0, 64)]),
                    op=ALU.is_equal)
                nc.vector.tensor_tensor(out=emask[:], in0=emask[:], in1=gw[:],
                                        op=ALU.mult)
                nc.vector.tensor_reduce(
                    out=wg[:, 64 * c:64 * (c + 1)],
                    in_=v(emask[:], 0, [(64, 64), (1, 64)]),
                    axis=AXL.X, op=ALU.add)

            # ---------------- mask & weights ----------------
            iota_k = pool.tile([P, NSLOT], I32)
            with tc.high_priority():
                nc.gpsimd.iota(iota_k[:].rearrange("p (q k) -> p q k", k=K),
                               pattern=[[0, PPT], [1, K]], base=0,
                               channel_multiplier=0)
            iota_kf = pool.tile([P, NSLOT], F32)
            nc.vector.tensor_copy(out=iota_kf[:], in_=iota_k[:])
            mask = pool.tile([P, NSLOT], F32)
            nc.vector.tensor_tensor(
                out=mask[:], in0=iota_kf[:],
                in1=v(misc_sb[:], 384, [(1, PPT), (0, K)]),
                op=ALU.is_lt)
            wm = pool.tile([P, NSLOT], F32)
            nc.vector.scalar_tensor_tensor(
                out=wm[:], in0=wg[:], scalar=1e-4, in1=mask[:],
                op0=ALU.mult, op1=ALU.mult)

            # ---------------- edge vectors (c, pt, k) ----------------
            e1 = pool.tile([P, 3 * NSLOT], F32)
            e2 = pool.tile([P, 3 * NSLOT], F32)
            for (e, ooff, goff) in ((e1, 400, 0), (e2, 448, 3)):
                nc.vector.tensor_tensor(
                    out=v(e[:], 0, [(NSLOT, 3), (K, PPT), (1, K)]),
                    in0=v(misc_sb[:], ooff, [(1, 3), (3, PPT), (0, K)]),
                    in1=v(gtab[:], goff, [(1, 3), (8 * K, PPT), (8, K)]),
                    op=ALU.subtract)

            # ---------------- A0 = S^T ----------------
            we1 = pool.tile([P, 3 * NSLOT], F32)
            nc.vector.tensor_tensor(
                out=we1[:], in0=e1[:],
                in1=v(wm[:], 0, [(0, 3), (1, NSLOT)]),
                op=ALU.mult)
            sprod = pool.tile([P, 9 * NSLOT], F32)
            nc.vector.tensor_tensor(
                out=sprod[:],
                in0=v(we1[:], 0, [(0, 3), (NSLOT, 3), (1, NSLOT)]),
                in1=v(e2[:], 0, [(NSLOT, 3), (0, 3), (1, NSLOT)]),
                op=ALU.mult)
            a0 = pool.tile([P, 9 * PPT], F32)
            nc.vector.tensor_reduce(
                out=v(a0[:], 0, [(PPT, 9), (1, PPT)]),
                in_=v(sprod[:], 0, [(NSLOT, 9), (K, PPT), (1, K)]),
                axis=AXL.X, op=ALU.add)
            nc.vector.tensor_scalar_add(
                v(a0[:], 0, [(4 * PPT, 3), (1, PPT)]),
                v(a0[:], 0, [(4 * PPT, 3), (1, PPT)]), EPS_S)

            # ---------------- Frobenius normalize -> X0 ----------------
            fprod = pool.tile([P, 9 * PPT], F32)
            nc.vector.tensor_tensor(out=fprod[:], in0=a0[:], in1=a0[:], op=ALU.mult)
            fro2 = pool.tile([P, PPT], F32)
            nc.vector.tensor_reduce(
                out=fro2[:], in_=v(fprod[:], 0, [(1, PPT), (PPT, 9)]),
                axis=AXL.X, op=ALU.add)
            fro = pool.tile([P, PPT], F32)
            nc.scalar.activation(fro[:], fro2[:], ACT.Sqrt, bias=0.0)
            rinv = pool.tile([P, PPT], F32)
            nc.vector.reciprocal(rinv[:], fro[:])
            x0 = pool.tile([P, 9 * PPT], F32)
            nc.vector.tensor_tensor(
                out=x0[:], in0=a0[:],
                in1=v(rinv[:], 0, [(0, 9), (1, PPT)]),
                op=ALU.mult)

            # ---------------- const tiles ----------------
            const_aI = pool.tile([P, 9 * PPT], F32)
            const_bI = pool.tile([P, 9 * PPT], F32)
            with tc.high_priority():
                nc.gpsimd.memset(const_aI[:], 0.0)
                nc.gpsimd.memset(v(const_aI[:], 0, [(4 * PPT, 3), (1, PPT)]), QA)
                nc.gpsimd.memset(const_bI[:], 0.0)
                nc.gpsimd.memset(v(const_bI[:], 0, [(4 * PPT, 3), (1, PPT)]), 1.5)

            # ---------------- Newton-Schulz ----------------
            xa = pool.tile([P, 9 * PPT], F32)
            xb = pool.tile([P, 9 * PPT], F32)
            mm = pool.tile([P, 9 * PPT], F32)
            m2 = pool.tile([P, 9 * PPT], F32)
            pp = pool.tile([P, 9 * PPT], F32)
            prod = pool.tile([P, 27 * PPT], F32)

            def mm_TN(dst, lhs, rhs):
                nc.vector.tensor_tensor(
                    out=prod[:],
                    in0=v(lhs, 0, [(3 * PPT, 3), (0, 3), (PPT, 3), (1, PPT)]),
                    in1=v(rhs, 0, [(3 * PPT, 3), (PPT, 3), (0, 3), (1, PPT)]),
                    op=ALU.mult)
                nc.vector.tensor_reduce(
                    out=v(dst, 0, [(3 * PPT, 3), (PPT, 3), (1, PPT)]),
                    in_=v(prod[:], 0, [(PPT, 3), (3 * PPT, 3), (1, PPT), (9 * PPT, 3)]),
                    axis=AXL.X, op=ALU.add)

            def mm_NN(dst, lhs, rhs):
                nc.vector.tensor_tensor(
                    out=prod[:],
                    in0=v(lhs, 0, [(3 * PPT, 3), (PPT, 3), (0, 3), (1, PPT)]),
                    in1=v(rhs, 0, [(0, 3), (3 * PPT, 3), (PPT, 3), (1, PPT)]),
                    op=ALU.mult)
                nc.vector.tensor_reduce(
                    out=v(dst, 0, [(3 * PPT, 3), (PPT, 3), (1, PPT)]),
                    in_=v(prod[:], 0, [(9 * PPT, 3), (PPT, 3), (1, PPT), (3 * PPT, 3)]),
                    axis=AXL.X, op=ALU.add)

            cur, nxt = x0, xa
            first = True
            for it in range(N_QUINTIC + N_CUBIC):
                mm_TN(mm[:], cur[:], cur[:])
                if it < N_QUINTIC:
                    mm_NN(m2[:], mm[:], mm[:])
                    nc.vector.scalar_tensor_tensor(
                        out=pp[:], in0=m2[:], scalar=QC, in1=const_aI[:],
                        op0=ALU.mult, op1=ALU.add)
                    nc.vector.scalar_tensor_tensor(
                        out=pp[:], in0=mm[:], scalar=QB, in1=pp[:],
                        op0=ALU.mult, op1=ALU.add)
                else:
                    nc.vector.scalar_tensor_tensor(
                        out=pp[:], in0=mm[:], scalar=-0.5, in1=const_bI[:],
                        op0=ALU.mult, op1=ALU.add)
                mm_NN(nxt[:], cur[:], pp[:])
                if first:
                    cur, nxt, first = nxt, xb, False
                else:
                    cur, nxt = nxt, cur
            w_q = cur

            # ---------------- det(Q) & gate ----------------
            r12 = pool.tile([P, 2 * 6 * PPT], F32)
            for half in range(2):
                nc.scalar.copy(
                    v(r12[:], half * 3 * PPT, [(6 * PPT, 2), (PPT, 3), (1, PPT)]),
                    v(w_q[:], 3 * PPT, [(3 * PPT, 2), (PPT, 3), (1, PPT)]))
            c0 = pool.tile([P, 3 * PPT], F32)
            t1 = pool.tile([P, 3 * PPT], F32)
            nc.vector.tensor_tensor(
                out=t1[:],
                in0=v(r12[:], 1 * PPT, [(PPT, 3), (1, PPT)]),
                in1=v(r12[:], 6 * PPT + 2 * PPT, [(PPT, 3), (1, PPT)]),
                op=ALU.mult)
            nc.vector.tensor_tensor(
                out=c0[:],
                in0=v(r12[:], 2 * PPT, [(PPT, 3), (1, PPT)]),
                in1=v(r12[:], 6 * PPT + 1 * PPT, [(PPT, 3), (1, PPT)]),
                op=ALU.mult)
            nc.vector.tensor_tensor(out=c0[:], in0=t1[:], in1=c0[:], op=ALU.subtract)
            dprod = pool.tile([P, 3 * PPT], F32)
            nc.vector.tensor_tensor(
                out=v(dprod[:], 0, [(3, PPT), (1, 3)]),
                in0=v(w_q[:], 0, [(1, PPT), (PPT, 3)]),
                in1=v(c0[:], 0, [(1, PPT), (PPT, 3)]),
                op=ALU.mult)
            det = pool.tile([P, PPT], F32)
            nc.vector.tensor_reduce(
                out=det[:], in_=v(dprod[:], 0, [(3, PPT), (1, 3)]),
                axis=AXL.X, op=ALU.add)
            gate = pool.tile([P, PPT], F32)
            nc.vector.tensor_scalar(out=gate[:], in0=det[:], scalar1=0.0,
                                    scalar2=2.0, op0=ALU.is_lt, op1=ALU.mult)

            # ---------------- H = Q^T X0, adj, power iteration ----------
            h = pool.tile([P, 9 * PPT], F32)
            mm_TN(h[:], w_q[:], x0[:])
            h2 = pool.tile([P, 36 * PPT], F32)
            for io in range(2):
                for jo in range(2):
                    nc.scalar.copy(
                        v(h2[:], (io * 3 * 6 + jo * 3) * PPT,
                          [(6 * PPT, 3), (PPT, 3), (1, PPT)]),
                        v(h[:], 0, [(3 * PPT, 3), (PPT, 3), (1, PPT)]))
            adjh = pool.tile([P, 9 * PPT], F32)
            tt = pool.tile([P, 9 * PPT], F32)
            nc.vector.tensor_tensor(
                out=tt[:],
                in0=v(h2[:], (6 + 1) * PPT, [(6 * PPT, 3), (PPT, 3), (1, PPT)]),
                in1=v(h2[:], (12 + 2) * PPT, [(6 * PPT, 3), (PPT, 3), (1, PPT)]),
                op=ALU.mult)
            nc.vector.tensor_tensor(
                out=adjh[:],
                in0=v(h2[:], (6 + 2) * PPT, [(6 * PPT, 3), (PPT, 3), (1, PPT)]),
                in1=v(h2[:], (12 + 1) * PPT, [(6 * PPT, 3), (PPT, 3), (1, PPT)]),
                op=ALU.mult)
            nc.vector.tensor_tensor(out=adjh[:], in0=tt[:], in1=adjh[:],
                                    op=ALU.subtract)

            va = pool.tile([P, 3 * PPT], F32)
            vb = pool.tile([P, 3 * PPT], F32)
            vprod = pool.tile([P, 9 * PPT], F32)
            with tc.high_priority():
                nc.gpsimd.memset(v(va[:], 0 * PPT, [(1, PPT)]), 0.5377)
                nc.gpsimd.memset(v(va[:], 1 * PPT, [(1, PPT)]), -0.3677)
                nc.gpsimd.memset(v(va[:], 2 * PPT, [(1, PPT)]), 0.7607)
            cv, nv = va, vb
            for _ in range(N_POWER):
                nc.vector.tensor_tensor(
                    out=v(vprod[:], 0, [(3 * PPT, 3), (3, PPT), (1, 3)]),
                    in0=v(adjh[:], 0, [(3 * PPT, 3), (1, PPT), (PPT, 3)]),
                    in1=v(cv[:], 0, [(0, 3), (1, PPT), (PPT, 3)]),
                    op=ALU.mult)
                nc.vector.tensor_reduce(
                    out=v(nv[:], 0, [(PPT, 3), (1, PPT)]),
                    in_=v(vprod[:], 0, [(3 * PPT, 3), (3, PPT), (1, 3)]),
                    axis=AXL.X, op=ALU.add)
                cv, nv = nv, cv
            n2p = pool.tile([P, 3 * PPT], F32)
            nc.vector.tensor_tensor(
                out=v(n2p[:], 0, [(3, PPT), (1, 3)]),
                in0=v(cv[:], 0, [(1, PPT), (PPT, 3)]),
                in1=v(cv[:], 0, [(1, PPT), (PPT, 3)]),
                op=ALU.mult)
            n2 = pool.tile([P, PPT], F32)
            nc.vector.tensor_reduce(
                out=n2[:], in_=v(n2p[:], 0, [(3, PPT), (1, 3)]),
                axis=AXL.X, op=ALU.add)
            nc.vector.tensor_scalar_max(n2[:], n2[:], 1e-30)
            nn_t = pool.tile([P, PPT], F32)
            nc.scalar.activation(nn_t[:], n2[:], ACT.Sqrt, bias=0.0)
            rn = pool.tile([P, PPT], F32)
            nc.vector.reciprocal(rn[:], nn_t[:])
            u3 = pool.tile([P, 3 * PPT], F32)
            nc.vector.tensor_tensor(
                out=u3[:], in0=cv[:],
                in1=v(rn[:], 0, [(0, 3), (1, PPT)]),
                op=ALU.mult)
            qu3 = pool.tile([P, 3 * PPT], F32)
            nc.vector.tensor_tensor(
                out=v(vprod[:], 0, [(3 * PPT, 3), (3, PPT), (1, 3)]),
                in0=v(w_q[:], 0, [(3 * PPT, 3), (1, PPT), (PPT, 3)]),
                in1=v(u3[:], 0, [(0, 3), (1, PPT), (PPT, 3)]),
                op=ALU.mult)
            nc.vector.tensor_reduce(
                out=v(qu3[:], 0, [(PPT, 3), (1, PPT)]),
                in_=v(vprod[:], 0, [(3 * PPT, 3), (3, PPT), (1, 3)]),
                axis=AXL.X, op=ALU.add)
            outer = pool.tile([P, 9 * PPT], F32)
            nc.vector.tensor_tensor(
                out=outer[:],
                in0=v(qu3[:], 0, [(PPT, 3), (0, 3), (1, PPT)]),
                in1=v(u3[:], 0, [(0, 3), (PPT, 3), (1, PPT)]),
                op=ALU.mult)
            nc.vector.tensor_tensor(
                out=outer[:], in0=outer[:],
                in1=v(gate[:], 0, [(0, 9), (1, PPT)]),
                op=ALU.mult)
            rmat = pool.tile([P, 9 * PPT], F32)
            nc.vector.tensor_tensor(out=rmat[:], in0=w_q[:], in1=outer[:],
                                    op=ALU.subtract)

            # ---------------- energies ----------------
            rprod = pool.tile([P, 9 * NSLOT], F32)
            nc.vector.tensor_tensor(
                out=rprod[:],
                in0=v(rmat[:], 0, [(1, 9 * PPT), (0, K)]),
                in1=v(e1[:], 0, [(0, 3), (NSLOT, 3), (1, NSLOT)]),
                op=ALU.mult)
            re1 = pool.tile([P, 3 * NSLOT], F32)
            nc.vector.tensor_reduce(
                out=v(re1[:], 0, [(NSLOT, 3), (K, PPT), (1, K)]),
                in_=v(rprod[:], 0, [(3 * NSLOT, 3), (K, PPT), (1, K), (NSLOT, 3)]),
                axis=AXL.X, op=ALU.add)
            resid = pool.tile([P, 3 * NSLOT], F32)
            nc.vector.tensor_tensor(out=resid[:], in0=e2[:], in1=re1[:],
                                    op=ALU.subtract)
            rsq = pool.tile([P, 3 * NSLOT], F32)
            nc.vector.tensor_tensor(out=rsq[:], in0=resid[:], in1=resid[:],
                                    op=ALU.mult)
            nc.vector.tensor_tensor(
                out=rsq[:], in0=rsq[:],
                in1=v(wm[:], 0, [(0, 3), (1, NSLOT)]),
                op=ALU.mult)
            energy = pool.tile([P, PPT * 3], F32)
            nc.vector.tensor_reduce(
                out=v(energy[:], 0, [(1, 3), (3, PPT)]),
                in_=v(rsq[:], 0, [(NSLOT, 3), (K, PPT), (1, K)]),
                axis=AXL.X, op=ALU.add)

            nc.sync.dma_start(out[:], energy[:])
    return nc


def build_compiled():
    nc = bacc.Bacc("TRN2", target_bir_lowering=False, debug=False,
                   num_devices=NCORES, num_swdge_queues=4)
    build(nc)
    nc.compile()
    return nc


def _wrap16(vals, nidx, reps):
    """vals[i] -> wrapped [16, nidx//16] (idx i at [i%16, i//16]), tiled."""
    w = np.zeros((16, nidx // 16), np.int16)
    w[np.arange(nidx) % 16, np.arange(nidx) // 16] = vals
    return np.tile(w, (reps, 1))


def shard_inputs(xyz1, xyz2, neighborList, numNeighbors, weightMatrix):
    # 6 coordinate planes [6, 16384]: plane j, vertex b*8192+n
    planes6 = np.concatenate(
        [xyz1.transpose(2, 0, 1).reshape(3, B * N),
         xyz2.transpose(2, 0, 1).reshape(3, B * N)], axis=0
    ).astype(np.float32)
    planes = np.ascontiguousarray(planes6)
    ident = np.eye(P, dtype=np.float32)

    pt = np.arange(PTS)
    q_of = pt >> 7
    p_of = pt & 127
    b_of = pt // RPC
    kk = np.arange(K)
    # global gather order: i = (q*16+k)*128 + p
    i_of = (q_of[:, None] * 16 + kk[None, :]) * 128 + p_of[:, None]  # [PTS,K]

    maps = []
    for c in range(NCORES):
        sl = slice(c * RPC, (c + 1) * RPC)
        nbrf = np.ascontiguousarray(neighborList[:, sl].reshape(PTS, K)).astype(np.int64)
        # ap_gather idx: per 16-partition group g, slots q in {2g, 2g+1},
        # local i' = ((q&1)*16+k)*128 + p, value = b*8192 + n
        cval = (b_of[:, None] * N + nbrf).astype(np.int64)
        aidx = np.zeros((P, NSLOT), np.int16)
        for g in range(8):
            lv = np.zeros(4096, np.int64)
            m = (q_of >> 1) == g
            ip = ((q_of[m][:, None] & 1) * 16 + kk[None, :]) * 128 + p_of[m][:, None]
            lv[ip.ravel()] = cval[m].ravel()
            aidx[16 * g:16 * (g + 1)] = _wrap16(lv.astype(np.int16), 4096, 1)
        # weight window idx: value = p*128 + (n>>6), order i
        wval = (p_of[:, None] * 128 + (nbrf >> 6)).astype(np.int64)
        lw = np.zeros(P * NSLOT, np.int64)
        lw[i_of.ravel()] = wval.ravel()
        widx = _wrap16(lw.astype(np.int16), P * NSLOT, 8)
        # n63 [p, slot=(q,k)] = n & 63 as f32
        l63 = np.zeros(P * NSLOT, np.int64)
        l63[i_of.ravel()] = (nbrf & 63).ravel()
        n63 = l63.reshape(NSLOT, P).T.astype(np.float32)
        n63 = np.ascontiguousarray(n63)
        numnf = numNeighbors[:, sl].reshape(PTS).reshape(PPT, P).T.astype(np.float32)
        o1 = xyz1[:, sl].reshape(PTS, 3).reshape(PPT, P, 3) \
            .transpose(1, 0, 2).reshape(P, PPT * 3).astype(np.float32)
        o2 = xyz2[:, sl].reshape(PTS, 3).reshape(PPT, P, 3) \
            .transpose(1, 0, 2).reshape(P, PPT * 3).astype(np.float32)
        pad = np.zeros((P, 16), np.float32)
        misc = np.ascontiguousarray(np.concatenate(
            [n63, ident, numnf, pad[:, :0], o1, o2], axis=1))
        assert misc.shape == (P, 496), misc.shape
        maps.append({
            "planes": planes,
            "aidx": aidx,
            "widx": widx,
            "misc": misc,
            "wmat": np.ascontiguousarray(weightMatrix[:, sl].reshape(PTS, N)).astype(np.float32),
        })
    return maps


def unshard_output(results):
    full = np.zeros((B, N, 3), dtype=np.float32)
    for c in range(NCORES):
        sl = slice(c * RPC, (c + 1) * RPC)
        arr = results[c]["out"].reshape(P, PPT, 3).transpose(1, 0, 2)
        full[:, sl] = arr.reshape(B, RPC, 3)
    return full


_NC_CACHE = None
LAST_EXEC_TIME_NS = None


def _get_nc():
    global _NC_CACHE
    if _NC_CACHE is None:
        _NC_CACHE = build_compiled()
    return _NC_CACHE


def _maybe_install_ntff_shim():
    import sys, types
    try:
        if "antenv.axon_hooks" not in sys.modules:
            mod = types.ModuleType("antenv.axon_hooks")
            mod._hook = None
            mod.set_axon_ntff_profile_hook = lambda h: setattr(mod, "_hook", h)
            mod.get_axon_ntff_profile_hook = lambda: mod._hook
            sys.modules["antenv.axon_hooks"] = mod
            import antenv
            antenv.axon_hooks = mod
            from trn_agent_boot.trn_boot import _ntff_profile_via_ctypes
            mod.set_axon_ntff_profile_hook(
                _ntff_profile_via_ctypes("/opt/axon/libaxon_pjrt.so"))
        return True
    except Exception:
        return False


def kernel(xyz1, xyz2, neighborList, numNeighbors, weightMatrix):
    """Full unsharded inputs -> full [2, 8192, 3] float32 output."""
    global LAST_EXEC_TIME_NS
    import os
    from concourse.bass_utils import run_bass_kernel_spmd
    nc = _get_nc()
    in_maps = shard_inputs(np.asarray(xyz1), np.asarray(xyz2),
                           np.asarray(neighborList), np.asarray(numNeighbors),
                           np.asarray(weightMatrix))
    trace = bool(os.environ.get("ARAP_TRACE")) and _maybe_install_ntff_shim()
    try:
        res = run_bass_kernel_spmd(nc, in_maps, core_ids=list(range(NCORES)),
                                   trace=trace)
    except Exception:
        if not trace:
            raise
        res = run_bass_kernel_spmd(nc, in_maps, core_ids=list(range(NCORES)))
    LAST_EXEC_TIME_NS = res.exec_time_ns
    return unshard_output(res.results)


# revision 31
# speedup vs baseline: 1.0205x; 1.0205x over previous
"""ARAP cell-energy Bass kernel (per-core SPMD program), v2.

Problem: B=2, N=8192, K=16. 8 cores; core c owns rows [c*1024,(c+1)*1024)
of BOTH batches -> 2048 points/core. Point mapping: pt = q*128 + p
(q in [0,16) slot-group, p = partition). Natural shard row order == pt.

Gather strategy (v2):
  - coords: host uploads 6 coordinate "planes" [6,16384] f32; device
    replicates them across all 128 partitions (partition 16g+j holds
    plane j), then ONE gpsimd.ap_gather fetches all 32768 neighbor
    coords (per-16-partition-group index lists, ~0.4us), and 32 PE
    transposes + DVE copies land them point-major in gtab (8-f32 slots).
  - weights: 32 x dma_gather (1024 idx each, 4 SWDGE queues -> 4x
    faster descriptor gen) fetch 256B windows wmat[pt, (n>>6)*64:+64];
    a one-hot (iota==n%64) mask + multiply + reduce extracts w[pt,n].
  - rotation: Frobenius-normalized Newton-Schulz, 6 quintic + 3 cubic
    iterations, det flip via 2-step power iteration on adj(Q^T X0).

Host-side prep in shard_inputs() is pure layout/index arithmetic
(transposes, reshapes, int16 casts, shifts); all data movement and
compute stay on device.
"""
import numpy as np
import concourse.bass as bass
import concourse.bacc as bacc
import concourse.tile as tile
import concourse.mybir as mybir
from concourse import ap_utils
from concourse.bass import exact_div

F32 = mybir.dt.float32
I32 = mybir.dt.int32
I16 = mybir.dt.int16
ALU = mybir.AluOpType
ACT = mybir.ActivationFunctionType
AXL = mybir.AxisListType

B, N, K = 2, 8192, 16
NCORES = 8
RPC = N // NCORES          # rows per core per batch = 1024
PTS = B * RPC              # points per core = 2048
P = 128                    # partitions
PPT = PTS // P             # slot-groups q = 16
NSLOT = PPT * K            # slots per partition = 256
EPS_S = 1e-8 * 1e-4

QA, QB, QC = 3.4445, -4.7750, 2.0315
N_QUINTIC = 5
N_CUBIC = 3
N_POWER = 2

NW_INSTR = 32              # weight gather instructions (1024 idx each)
WCHUNK = 8                 # weight instrs per extraction chunk
NCHUNK = NW_INSTR // WCHUNK


def v(ap, off, dims):
    part = list(ap.ap[0])
    return bass.AP(ap.tensor, ap.offset + off, [part] + [[s, n] for s, n in dims])


def dma_gather_raw(gp, out_ap, in_ap, idxs_ap, num_idxs, elem_size, elem_step,
                   queue_num=0):
    """dma_gather without the elem_size%256 restriction (non-transpose,
    DRAM source)."""
    assert idxs_ap.dtype == I16
    assert in_ap.dtype == out_ap.dtype
    assert in_ap.space == bass.MemorySpace.DRAM
    assert ap_utils.ap_is_contiguous(out_ap.ap[1:])
    assert ap_utils.ap_is_contiguous(idxs_ap.ap[1:])
    assert in_ap.ap[0][0] == elem_step
    stride_bytes_256 = exact_div(elem_step * mybir.dt.size(in_ap.dtype), 256)
    _in_ap = gp.lower_ap_dma(in_ap, for_custom_bir_dma=True)
    return gp.add_instruction(
        mybir.InstDMAGatherAnt(
            name=gp.bass.get_next_instruction_name(),
            ins=[*_in_ap, gp.lower_ap(idxs_ap),
                 gp.lower_val_access(gp.to_reg(num_idxs))],
            outs=[gp.lower_ap(out_ap)],
            transpose=False, num_idxs=num_idxs, elem_size=elem_size,
            stride_bytes_256=stride_bytes_256, gen_mode=0, single_packet=True,
            queue_num=queue_num, sbuf_tokens_per_rank=0,
            sbuf_free_dim_per_rank=0, sbuf_free_dim_pad_per_rank=0,
            sbuf_byte_offset=0))


def build(nc):
    ctab = nc.declare_dram_parameter("ctab", [B * N + 64, 64], F32, isOutput=False)
    cidx = nc.declare_dram_parameter("cidx", [P, 2048], I16, isOutput=False)
    widx = nc.declare_dram_parameter("widx", [P, 2048], I16, isOutput=False)
    misc = nc.declare_dram_parameter("misc", [P, 496], F32, isOutput=False)
    wmat = nc.declare_dram_parameter("wmat", [PTS, N], F32, isOutput=False)
    out = nc.declare_dram_parameter("out", [P, PPT * 3], F32, isOutput=True)
    with tile.TileContext(nc) as tc:
        with tc.tile_pool(name="sb", bufs=1) as pool, \
             tc.tile_pool(name="gw", bufs=2) as gwpool, \
             tc.tile_pool(name="msk", bufs=1) as mpool, \
             tc.tile_pool(name="ps", bufs=8, space="PSUM") as pspool:
            # ---------------- small loads ----------------
            cidx_sb = pool.tile([P, 2048], I16)
            widx_sb = pool.tile([P, 2048], I16)
            misc_sb = pool.tile([P, 496], F32)
            nc.sync.dma_start(widx_sb[:], widx[:])
            nc.scalar.dma_start(cidx_sb[:], cidx[:])
            nc.scalar.dma_start(misc_sb[:], misc[:])
            n63_sb = misc_sb      # [:, 0:256]
            ident_sb = misc_sb    # [:, 256:384]
            own1_sb = misc_sb     # [:, 400:448]
            own2_sb = misc_sb     # [:, 448:496]

            # ------- coord + weight window gathers, interleaved ---------
            # Both use lib-3 dma_gather: no mid-kernel gpsimd library
            # reload (the old ap_gather path cost ~110us of Pool stall on
            # the lib-6 ucode swap). Coords: 8-f32 windows (elem-size
            # bypass, 256B-pitch ctab) land directly in gtab layout.
            in_c = bass.AP(ctab[:].tensor, 0, [[64, B * N + 64], [1, 8]])
            gtab = pool.tile([P, NSLOT * 8], F32)
            gw_tiles = []
            emit = 0
            for c in range(NCHUNK):
                gw = gwpool.tile([P, WCHUNK * 8 * 64], F32)
                gw_tiles.append(gw)
                for t in range(WCHUNK):
                    j = c * WCHUNK + t
                    cg_ap = bass.AP(gtab[:].tensor, gtab[:].offset + 64 * j,
                                    [list(gtab[:].ap[0]), [8, 8], [1, 8]])
                    dma_gather_raw(nc.gpsimd, cg_ap, in_c,
                                   cidx_sb[:, 64 * j:64 * (j + 1)],
                                   1024, 8, 64, queue_num=emit % 4)
                    emit += 1
                    q = j // 2
                    in_ap = bass.AP(wmat[:].tensor, q * 128 * N,
                                    [[64, 128 * 128], [1, 64]])
                    out_ap = bass.AP(gw[:].tensor, gw[:].offset + t * 8 * 64,
                                     [list(gw[:].ap[0]), [64, 8], [1, 64]])
                    nc.gpsimd.dma_gather(out_ap, in_ap,
                                         widx_sb[:, 64 * j:64 * (j + 1)],
                                         1024, 1024, 64, queue_num=emit % 4)
                    emit += 1

            # ---------------- weight extraction ----------------
            iota64i = pool.tile([P, 64], I32)
            with tc.high_priority():
                nc.gpsimd.iota(iota64i[:], pattern=[[1, 64]], base=0,
                               channel_multiplier=0)
            iota64 = pool.tile([P, 64], F32)
            nc.vector.tensor_copy(out=iota64[:], in_=iota64i[:])
            wg = pool.tile([P, NSLOT], F32)
            for c in range(NCHUNK):
                gw = gw_tiles[c]
                emask = mpool.tile([P, 64 * 64], F32)
                nc.vector.tensor_tensor(
                    out=emask[:],
                    in0=v(iota64[:], 0, [(0, 64), (1, 64)]),
                    in1=v(n63_sb[:], 64 * c, [(1, 64), (0, 64)]),
                    op=ALU.is_equal)
                nc.vector.tensor_tensor(out=emask[:], in0=emask[:], in1=gw[:],
                                        op=ALU.mult)
                nc.vector.tensor_reduce(
                    out=wg[:, 64 * c:64 * (c + 1)],
                    in_=v(emask[:], 0, [(64, 64), (1, 64)]),
                    axis=AXL.X, op=ALU.add)

            # ---------------- mask & weights ----------------
            iota_k = pool.tile([P, NSLOT], I32)
            with tc.high_priority():
                nc.gpsimd.iota(iota_k[:].rearrange("p (q k) -> p q k", k=K),
                               pattern=[[0, PPT], [1, K]], base=0,
                               channel_multiplier=0)
            iota_kf = pool.tile([P, NSLOT], F32)
            nc.vector.tensor_copy(out=iota_kf[:], in_=iota_k[:])
            mask = pool.tile([P, NSLOT], F32)
            nc.vector.tensor_tensor(
                out=mask[:], in0=iota_kf[:],
                in1=v(misc_sb[:], 384, [(1, PPT), (0, K)]),
                op=ALU.is_lt)
            wm = pool.tile([P, NSLOT], F32)
            nc.vector.scalar_tensor_tensor(
                out=wm[:], in0=wg[:], scalar=1e-4, in1=mask[:],
                op0=ALU.mult, op1=ALU.mult)

            # ---------------- edge vectors (c, pt, k) ----------------
            e1 = pool.tile([P, 3 * NSLOT], F32)
            e2 = pool.tile([P, 3 * NSLOT], F32)
            for (e, ooff, goff) in ((e1, 400, 0), (e2, 448, 3)):
                nc.vector.tensor_tensor(
                    out=v(e[:], 0, [(NSLOT, 3), (K, PPT), (1, K)]),
                    in0=v(misc_sb[:], ooff, [(1, 3), (3, PPT), (0, K)]),
                    in1=v(gtab[:], goff, [(1, 3), (8 * K, PPT), (8, K)]),
                    op=ALU.subtract)

            # ---------------- A0 = S^T ----------------
            we1 = pool.tile([P, 3 * NSLOT], F32)
            nc.vector.tensor_tensor(
                out=we1[:], in0=e1[:],
                in1=v(wm[:], 0, [(0, 3), (1, NSLOT)]),
                op=ALU.mult)
            sprod = pool.tile([P, 9 * NSLOT], F32)
            nc.vector.tensor_tensor(
                out=sprod[:],
                in0=v(we1[:], 0, [(0, 3), (NSLOT, 3), (1, NSLOT)]),
                in1=v(e2[:], 0, [(NSLOT, 3), (0, 3), (1, NSLOT)]),
                op=ALU.mult)
            a0 = pool.tile([P, 9 * PPT], F32)
            nc.vector.tensor_reduce(
                out=v(a0[:], 0, [(PPT, 9), (1, PPT)]),
                in_=v(sprod[:], 0, [(NSLOT, 9), (K, PPT), (1, K)]),
                axis=AXL.X, op=ALU.add)
            nc.vector.tensor_scalar_add(
                v(a0[:], 0, [(4 * PPT, 3), (1, PPT)]),
                v(a0[:], 0, [(4 * PPT, 3), (1, PPT)]), EPS_S)

            # ---------------- Frobenius normalize -> X0 ----------------
            fprod = pool.tile([P, 9 * PPT], F32)
            nc.vector.tensor_tensor(out=fprod[:], in0=a0[:], in1=a0[:], op=ALU.mult)
            fro2 = pool.tile([P, PPT], F32)
            nc.vector.tensor_reduce(
                out=fro2[:], in_=v(fprod[:], 0, [(1, PPT), (PPT, 9)]),
                axis=AXL.X, op=ALU.add)
            fro = pool.tile([P, PPT], F32)
            nc.scalar.activation(fro[:], fro2[:], ACT.Sqrt, bias=0.0)
            rinv = pool.tile([P, PPT], F32)
            nc.vector.reciprocal(rinv[:], fro[:])
            x0 = pool.tile([P, 9 * PPT], F32)
            nc.vector.tensor_tensor(
                out=x0[:], in0=a0[:],
                in1=v(rinv[:], 0, [(0, 9), (1, PPT)]),
                op=ALU.mult)

            # ---------------- const tiles ----------------
            const_aI = pool.tile([P, 9 * PPT], F32)
            const_bI = pool.tile([P, 9 * PPT], F32)
            with tc.high_priority():
                nc.gpsimd.memset(const_aI[:], 0.0)
                nc.gpsimd.memset(v(const_aI[:], 0, [(4 * PPT, 3), (1, PPT)]), QA)
                nc.gpsimd.memset(const_bI[:], 0.0)
                nc.gpsimd.memset(v(const_bI[:], 0, [(4 * PPT, 3), (1, PPT)]), 1.5)

            # ---------------- Newton-Schulz ----------------
            xa = pool.tile([P, 9 * PPT], F32)
            xb = pool.tile([P, 9 * PPT], F32)
            mm = pool.tile([P, 9 * PPT], F32)
            m2 = pool.tile([P, 9 * PPT], F32)
            pp = pool.tile([P, 9 * PPT], F32)
            prod = pool.tile([P, 27 * PPT], F32)

            def mm_TN(dst, lhs, rhs):
                nc.vector.tensor_tensor(
                    out=prod[:],
                    in0=v(lhs, 0, [(3 * PPT, 3), (0, 3), (PPT, 3), (1, PPT)]),
                    in1=v(rhs, 0, [(3 * PPT, 3), (PPT, 3), (0, 3), (1, PPT)]),
                    op=ALU.mult)
                nc.vector.tensor_reduce(
                    out=v(dst, 0, [(3 * PPT, 3), (PPT, 3), (1, PPT)]),
                    in_=v(prod[:], 0, [(PPT, 3), (3 * PPT, 3), (1, PPT), (9 * PPT, 3)]),
                    axis=AXL.X, op=ALU.add)

            def mm_NN(dst, lhs, rhs):
                nc.vector.tensor_tensor(
                    out=prod[:],
                    in0=v(lhs, 0, [(3 * PPT, 3), (PPT, 3), (0, 3), (1, PPT)]),
                    in1=v(rhs, 0, [(0, 3), (3 * PPT, 3), (PPT, 3), (1, PPT)]),
                    op=ALU.mult)
                nc.vector.tensor_reduce(
                    out=v(dst, 0, [(3 * PPT, 3), (PPT, 3), (1, PPT)]),
                    in_=v(prod[:], 0, [(9 * PPT, 3), (PPT, 3), (1, PPT), (3 * PPT, 3)]),
                    axis=AXL.X, op=ALU.add)

            cur, nxt = x0, xa
            first = True
            for it in range(N_QUINTIC + N_CUBIC):
                mm_TN(mm[:], cur[:], cur[:])
                if it < N_QUINTIC:
                    mm_NN(m2[:], mm[:], mm[:])
                    nc.vector.scalar_tensor_tensor(
                        out=pp[:], in0=m2[:], scalar=QC, in1=const_aI[:],
                        op0=ALU.mult, op1=ALU.add)
                    nc.vector.scalar_tensor_tensor(
                        out=pp[:], in0=mm[:], scalar=QB, in1=pp[:],
                        op0=ALU.mult, op1=ALU.add)
                else:
                    nc.vector.scalar_tensor_tensor(
                        out=pp[:], in0=mm[:], scalar=-0.5, in1=const_bI[:],
                        op0=ALU.mult, op1=ALU.add)
                mm_NN(nxt[:], cur[:], pp[:])
                if first:
                    cur, nxt, first = nxt, xb, False
                else:
                    cur, nxt = nxt, cur
            w_q = cur

            # ---------------- det(Q) & gate ----------------
            r12 = pool.tile([P, 2 * 6 * PPT], F32)
            for half in range(2):
                nc.scalar.copy(
                    v(r12[:], half * 3 * PPT, [(6 * PPT, 2), (PPT, 3), (1, PPT)]),
                    v(w_q[:], 3 * PPT, [(3 * PPT, 2), (PPT, 3), (1, PPT)]))
            c0 = pool.tile([P, 3 * PPT], F32)
            t1 = pool.tile([P, 3 * PPT], F32)
            nc.vector.tensor_tensor(
                out=t1[:],
                in0=v(r12[:], 1 * PPT, [(PPT, 3), (1, PPT)]),
                in1=v(r12[:], 6 * PPT + 2 * PPT, [(PPT, 3), (1, PPT)]),
                op=ALU.mult)
            nc.vector.tensor_tensor(
                out=c0[:],
                in0=v(r12[:], 2 * PPT, [(PPT, 3), (1, PPT)]),
                in1=v(r12[:], 6 * PPT + 1 * PPT, [(PPT, 3), (1, PPT)]),
                op=ALU.mult)
            nc.vector.tensor_tensor(out=c0[:], in0=t1[:], in1=c0[:], op=ALU.subtract)
            dprod = pool.tile([P, 3 * PPT], F32)
            nc.vector.tensor_tensor(
                out=v(dprod[:], 0, [(3, PPT), (1, 3)]),
                in0=v(w_q[:], 0, [(1, PPT), (PPT, 3)]),
                in1=v(c0[:], 0, [(1, PPT), (PPT, 3)]),
                op=ALU.mult)
            det = pool.tile([P, PPT], F32)
            nc.vector.tensor_reduce(
                out=det[:], in_=v(dprod[:], 0, [(3, PPT), (1, 3)]),
                axis=AXL.X, op=ALU.add)
            gate = pool.tile([P, PPT], F32)
            nc.vector.tensor_scalar(out=gate[:], in0=det[:], scalar1=0.0,
                                    scalar2=2.0, op0=ALU.is_lt, op1=ALU.mult)

            # ---------------- H = Q^T X0, adj, power iteration ----------
            h = pool.tile([P, 9 * PPT], F32)
            mm_TN(h[:], w_q[:], x0[:])
            h2 = pool.tile([P, 36 * PPT], F32)
            for io in range(2):
                for jo in range(2):
                    nc.scalar.copy(
                        v(h2[:], (io * 3 * 6 + jo * 3) * PPT,
                          [(6 * PPT, 3), (PPT, 3), (1, PPT)]),
                        v(h[:], 0, [(3 * PPT, 3), (PPT, 3), (1, PPT)]))
            adjh = pool.tile([P, 9 * PPT], F32)
            tt = pool.tile([P, 9 * PPT], F32)
            nc.vector.tensor_tensor(
                out=tt[:],
                in0=v(h2[:], (6 + 1) * PPT, [(6 * PPT, 3), (PPT, 3), (1, PPT)]),
                in1=v(h2[:], (12 + 2) * PPT, [(6 * PPT, 3), (PPT, 3), (1, PPT)]),
                op=ALU.mult)
            nc.vector.tensor_tensor(
                out=adjh[:],
                in0=v(h2[:], (6 + 2) * PPT, [(6 * PPT, 3), (PPT, 3), (1, PPT)]),
                in1=v(h2[:], (12 + 1) * PPT, [(6 * PPT, 3), (PPT, 3), (1, PPT)]),
                op=ALU.mult)
            nc.vector.tensor_tensor(out=adjh[:], in0=tt[:], in1=adjh[:],
                                    op=ALU.subtract)

            va = pool.tile([P, 3 * PPT], F32)
            vb = pool.tile([P, 3 * PPT], F32)
            vprod = pool.tile([P, 9 * PPT], F32)
            with tc.high_priority():
                nc.gpsimd.memset(v(va[:], 0 * PPT, [(1, PPT)]), 0.5377)
                nc.gpsimd.memset(v(va[:], 1 * PPT, [(1, PPT)]), -0.3677)
                nc.gpsimd.memset(v(va[:], 2 * PPT, [(1, PPT)]), 0.7607)
            cv, nv = va, vb
            for _ in range(N_POWER):
                nc.vector.tensor_tensor(
                    out=v(vprod[:], 0, [(3 * PPT, 3), (3, PPT), (1, 3)]),
                    in0=v(adjh[:], 0, [(3 * PPT, 3), (1, PPT), (PPT, 3)]),
                    in1=v(cv[:], 0, [(0, 3), (1, PPT), (PPT, 3)]),
                    op=ALU.mult)
                nc.vector.tensor_reduce(
                    out=v(nv[:], 0, [(PPT, 3), (1, PPT)]),
                    in_=v(vprod[:], 0, [(3 * PPT, 3), (3, PPT), (1, 3)]),
                    axis=AXL.X, op=ALU.add)
                cv, nv = nv, cv
            n2p = pool.tile([P, 3 * PPT], F32)
            nc.vector.tensor_tensor(
                out=v(n2p[:], 0, [(3, PPT), (1, 3)]),
                in0=v(cv[:], 0, [(1, PPT), (PPT, 3)]),
                in1=v(cv[:], 0, [(1, PPT), (PPT, 3)]),
                op=ALU.mult)
            n2 = pool.tile([P, PPT], F32)
            nc.vector.tensor_reduce(
                out=n2[:], in_=v(n2p[:], 0, [(3, PPT), (1, 3)]),
                axis=AXL.X, op=ALU.add)
            nc.vector.tensor_scalar_max(n2[:], n2[:], 1e-30)
            nn_t = pool.tile([P, PPT], F32)
            nc.scalar.activation(nn_t[:], n2[:], ACT.Sqrt, bias=0.0)
            rn = pool.tile([P, PPT], F32)
            nc.vector.reciprocal(rn[:], nn_t[:])
            u3 = pool.tile([P, 3 * PPT], F32)
            nc.vector.tensor_tensor(
                out=u3[:], in0=cv[:],
                in1=v(rn[:], 0, [(0, 3), (1, PPT)]),
                op=ALU.mult)
            qu3 = pool.tile([P, 3 * PPT], F32)
            nc.vector.tensor_tensor(
                out=v(vprod[:], 0, [(3 * PPT, 3), (3, PPT), (1, 3)]),
                in0=v(w_q[:], 0, [(3 * PPT, 3), (1, PPT), (PPT, 3)]),
                in1=v(u3[:], 0, [(0, 3), (1, PPT), (PPT, 3)]),
                op=ALU.mult)
            nc.vector.tensor_reduce(
                out=v(qu3[:], 0, [(PPT, 3), (1, PPT)]),
                in_=v(vprod[:], 0, [(3 * PPT, 3), (3, PPT), (1, 3)]),
                axis=AXL.X, op=ALU.add)
            outer = pool.tile([P, 9 * PPT], F32)
            nc.vector.tensor_tensor(
                out=outer[:],
                in0=v(qu3[:], 0, [(PPT, 3), (0, 3), (1, PPT)]),
                in1=v(u3[:], 0, [(0, 3), (PPT, 3), (1, PPT)]),
                op=ALU.mult)
            nc.vector.tensor_tensor(
                out=outer[:], in0=outer[:],
                in1=v(gate[:], 0, [(0, 9), (1, PPT)]),
                op=ALU.mult)
            rmat = pool.tile([P, 9 * PPT], F32)
            nc.vector.tensor_tensor(out=rmat[:], in0=w_q[:], in1=outer[:],
                                    op=ALU.subtract)

            # ---------------- energies ----------------
            rprod = pool.tile([P, 9 * NSLOT], F32)
            nc.vector.tensor_tensor(
                out=rprod[:],
                in0=v(rmat[:], 0, [(1, 9 * PPT), (0, K)]),
                in1=v(e1[:], 0, [(0, 3), (NSLOT, 3), (1, NSLOT)]),
                op=ALU.mult)
            re1 = pool.tile([P, 3 * NSLOT], F32)
            nc.vector.tensor_reduce(
                out=v(re1[:], 0, [(NSLOT, 3), (K, PPT), (1, K)]),
                in_=v(rprod[:], 0, [(3 * NSLOT, 3), (K, PPT), (1, K), (NSLOT, 3)]),
                axis=AXL.X, op=ALU.add)
            resid = pool.tile([P, 3 * NSLOT], F32)
            nc.vector.tensor_tensor(out=resid[:], in0=e2[:], in1=re1[:],
                                    op=ALU.subtract)
            rsq = pool.tile([P, 3 * NSLOT], F32)
            nc.vector.tensor_tensor(out=rsq[:], in0=resid[:], in1=resid[:],
                                    op=ALU.mult)
            nc.vector.tensor_tensor(
                out=rsq[:], in0=rsq[:],
                in1=v(wm[:], 0, [(0, 3), (1, NSLOT)]),
                op=ALU.mult)
            energy = pool.tile([P, PPT * 3], F32)
            nc.vector.tensor_reduce(
                out=v(energy[:], 0, [(1, 3), (3, PPT)]),
                in_=v(rsq[:], 0, [(NSLOT, 3), (K, PPT), (1, K)]),
                axis=AXL.X, op=ALU.add)

            nc.sync.dma_start(out[:], energy[:])
    return nc


def build_compiled():
    nc = bacc.Bacc("TRN2", target_bir_lowering=False, debug=False,
                   num_devices=NCORES, num_swdge_queues=4)
    build(nc)
    nc.compile()
    return nc


def _wrap16(vals, nidx, reps):
    """vals[i] -> wrapped [16, nidx//16] (idx i at [i%16, i//16]), tiled."""
    w = np.zeros((16, nidx // 16), np.int16)
    w[np.arange(nidx) % 16, np.arange(nidx) // 16] = vals
    return np.tile(w, (reps, 1))


def shard_inputs(xyz1, xyz2, neighborList, numNeighbors, weightMatrix):
    # 6 coordinate planes [6, 16384]: plane j, vertex b*8192+n
    ctab = np.zeros((B * N + 64, 64), np.float32)
    ctab[:B * N, 0:3] = xyz1.reshape(B * N, 3)
    ctab[:B * N, 3:6] = xyz2.reshape(B * N, 3)
    ident = np.eye(P, dtype=np.float32)

    pt = np.arange(PTS)
    q_of = pt >> 7
    p_of = pt & 127
    b_of = pt // RPC
    kk = np.arange(K)
    # global gather order: i = (q*16+k)*128 + p
    i_of = (q_of[:, None] * 16 + kk[None, :]) * 128 + p_of[:, None]  # [PTS,K]

    maps = []
    for c in range(NCORES):
        sl = slice(c * RPC, (c + 1) * RPC)
        nbrf = np.ascontiguousarray(neighborList[:, sl].reshape(PTS, K)).astype(np.int64)
        # ap_gather idx: per 16-partition group g, slots q in {2g, 2g+1},
        # local i' = ((q&1)*16+k)*128 + p, value = b*8192 + n
        cval = (b_of[:, None] * N + nbrf).astype(np.int64)
        lc = np.zeros(P * NSLOT, np.int64)
        lc[i_of.ravel()] = cval.ravel()
        cidx = _wrap16(lc.astype(np.int16), P * NSLOT, 8)
        # weight window idx: value = p*128 + (n>>6), order i
        wval = (p_of[:, None] * 128 + (nbrf >> 6)).astype(np.int64)
        lw = np.zeros(P * NSLOT, np.int64)
        lw[i_of.ravel()] = wval.ravel()
        widx = _wrap16(lw.astype(np.int16), P * NSLOT, 8)
        # n63 [p, slot=(q,k)] = n & 63 as f32
        l63 = np.zeros(P * NSLOT, np.int64)
        l63[i_of.ravel()] = (nbrf & 63).ravel()
        n63 = l63.reshape(NSLOT, P).T.astype(np.float32)
        n63 = np.ascontiguousarray(n63)
        numnf = numNeighbors[:, sl].reshape(PTS).reshape(PPT, P).T.astype(np.float32)
        o1 = xyz1[:, sl].reshape(PTS, 3).reshape(PPT, P, 3) \
            .transpose(1, 0, 2).reshape(P, PPT * 3).astype(np.float32)
        o2 = xyz2[:, sl].reshape(PTS, 3).reshape(PPT, P, 3) \
            .transpose(1, 0, 2).reshape(P, PPT * 3).astype(np.float32)
        pad = np.zeros((P, 16), np.float32)
        misc = np.ascontiguousarray(np.concatenate(
            [n63, ident, numnf, pad[:, :0], o1, o2], axis=1))
        assert misc.shape == (P, 496), misc.shape
        maps.append({
            "ctab": ctab,
            "cidx": cidx,
            "widx": widx,
            "misc": misc,
            "wmat": np.ascontiguousarray(weightMatrix[:, sl].reshape(PTS, N)).astype(np.float32),
        })
    return maps


def unshard_output(results):
    full = np.zeros((B, N, 3), dtype=np.float32)
    for c in range(NCORES):
        sl = slice(c * RPC, (c + 1) * RPC)
        arr = results[c]["out"].reshape(P, PPT, 3).transpose(1, 0, 2)
        full[:, sl] = arr.reshape(B, RPC, 3)
    return full


_NC_CACHE = None
LAST_EXEC_TIME_NS = None


def _get_nc():
    global _NC_CACHE
    if _NC_CACHE is None:
        _NC_CACHE = build_compiled()
    return _NC_CACHE


def _maybe_install_ntff_shim():
    import sys, types
    try:
        if "antenv.axon_hooks" not in sys.modules:
            mod = types.ModuleType("antenv.axon_hooks")
            mod._hook = None
            mod.set_axon_ntff_profile_hook = lambda h: setattr(mod, "_hook", h)
            mod.get_axon_ntff_profile_hook = lambda: mod._hook
            sys.modules["antenv.axon_hooks"] = mod
            import antenv
            antenv.axon_hooks = mod
            from trn_agent_boot.trn_boot import _ntff_profile_via_ctypes
            mod.set_axon_ntff_profile_hook(
                _ntff_profile_via_ctypes("/opt/axon/libaxon_pjrt.so"))
        return True
    except Exception:
        return False


def kernel(xyz1, xyz2, neighborList, numNeighbors, weightMatrix):
    """Full unsharded inputs -> full [2, 8192, 3] float32 output."""
    global LAST_EXEC_TIME_NS
    import os
    from concourse.bass_utils import run_bass_kernel_spmd
    nc = _get_nc()
    in_maps = shard_inputs(np.asarray(xyz1), np.asarray(xyz2),
                           np.asarray(neighborList), np.asarray(numNeighbors),
                           np.asarray(weightMatrix))
    trace = bool(os.environ.get("ARAP_TRACE")) and _maybe_install_ntff_shim()
    try:
        res = run_bass_kernel_spmd(nc, in_maps, core_ids=list(range(NCORES)),
                                   trace=trace)
    except Exception:
        if not trace:
            raise
        res = run_bass_kernel_spmd(nc, in_maps, core_ids=list(range(NCORES)))
    LAST_EXEC_TIME_NS = res.exec_time_ns
    return unshard_output(res.results)
